# revision 16
# baseline (speedup 1.0000x reference)
"""Trainium2 Bass kernel for nn_NeuralMemory (meta-memory injection + query conv
projection + l2norm + ResLinear MLP + sliding-window attention).

Strategy: shard the 2*2048 output tokens over 8 cores as (batch b, chunk c) with
c covering 512 query tokens. Each core recomputes the MLP for its 512 owned
tokens plus a 256-token left halo (the attention key window) -- no collectives.
All compute stays in "CP" layout (channels on partitions, tokens on free dim),
so every matmul uses the weights as lhsT exactly as stored and activations as
rhs; V is produced in token-partition layout by using the activations as lhsT.

v2: conv as PE diagonal matmuls, tt-pipelined l2norm, 768-wide MLP point ops on
2-bank PSUM tiles, head-pair software-pipelined attention, scoped PSUM pools.
"""
import os
import numpy as np
import ml_dtypes
from contextlib import ExitStack

import concourse.bass as bass
import concourse.tile as tile
from concourse import bacc, mybir
from concourse.bass_utils import run_bass_kernel_spmd
from concourse.masks import make_identity

AF = mybir.ActivationFunctionType
ALU = mybir.AluOpType
DT = mybir.dt
bf16 = ml_dtypes.bfloat16

B, S, D, HID, M = 2, 2048, 512, 1024, 64
K, NH, DH, WIN, L = 4, 8, 64, 256, 2
T = M + S                      # 2112
CHUNK = 512                    # owned query tokens per core
HALO = 384                     # left halo in X-token space (keys start at local 128)
XLEN = HALO + CHUNK            # 896
NKEY = 768                     # key positions per core (local tokens 128..896)
NQ = 512
P = 128
NCORES = 8

LAST_RESULTS = None


def _build_nc(repeat=1):
    nc = bacc.Bacc("TRN2", target_bir_lowering=False, debug=False,
                   enable_asserts=False)

    def din(name, shape, dt):
        return nc.dram_tensor(name, shape, dt, kind="ExternalInput").ap()

    xt = din("xt", (4, P, XLEN), DT.bfloat16)
    msk = din("msk", (P, 4, 3, P), DT.bfloat16)
    qp_w = din("qp_w", (4, P, D), DT.bfloat16)
    kdiag = din("kdiag", (4, K, P, P), DT.bfloat16)
    qp_b = din("qp_b", (P, 4), DT.float32)
    w_in = din("w_in", (4, P, HID), DT.bfloat16)
    b_in = din("b_in", (P, 8), DT.float32)
    w_hid = din("w_hid", (L, 8, P, HID), DT.bfloat16)
    b_hid = din("b_hid", (P, L, 8), DT.float32)
    w_out = din("w_out", (8, P, D), DT.bfloat16)
    b_out = din("b_out", (P, 4), DT.float32)
    wq = din("wq", (4, P, D), DT.bfloat16)
    bq = din("bq", (P, 4), DT.float32)
    wk = din("wk", (4, P, D), DT.bfloat16)
    bk = din("bk", (P, 4), DT.float32)
    wv = din("wv", (4, P, D), DT.bfloat16)
    bvb = din("bvb", (1, D), DT.bfloat16)
    wo = din("wo", (4, P, D), DT.bfloat16)
    bo = din("bo", (P, 4), DT.float32)
    out = nc.dram_tensor("out", (4, P, NQ), DT.float32, kind="ExternalOutput").ap()

    aps = {k: v for k, v in locals().items() if k not in ('nc', 'din')}
    with tile.TileContext(nc) as tc:
        for _ in range(repeat):
            with ExitStack() as ctx:
                _body(ctx, tc, aps)
    nc.finalize()
    return nc


def _body(ctx, tc, aps):
    nc = tc.nc
    f32 = DT.float32
    b16 = DT.bfloat16

    consts = ctx.enter_context(tc.tile_pool(name="consts", bufs=1))
    acts = ctx.enter_context(tc.tile_pool(name="acts", bufs=1))
    work = ctx.enter_context(tc.tile_pool(name="work", bufs=4))

    dma = nc.sync.dma_start
    mm = nc.tensor.matmul

    # ---- stream inputs into SBUF ----
    xt_sb = acts.tile([P, 4, XLEN], b16, name="xt_sb")
    qp_w_sb = consts.tile([P, 4, D], b16, name="qp_w_sb")
    for cc in range(4):
        dma(out=xt_sb[:, cc], in_=aps['xt'][cc])
        dma(out=qp_w_sb[:, cc], in_=aps['qp_w'][cc])
    kdiag_sb = consts.tile([P, 4, K, P], b16, name="kdiag_sb")
    for cc in range(4):
        dma(out=kdiag_sb[:, cc], in_=aps['kdiag'][cc].rearrange("k p n -> p k n"))
    qp_b_sb = consts.tile([P, 4], f32, name="qp_b_sb")
    dma(out=qp_b_sb, in_=aps['qp_b'])
    w_in_sb = consts.tile([P, 4, HID], b16, name="w_in_sb")
    for cc in range(4):
        dma(out=w_in_sb[:, cc], in_=aps['w_in'][cc])
    b_in_sb = consts.tile([P, 8], f32, name="b_in_sb")
    dma(out=b_in_sb, in_=aps['b_in'])
    w_hid_sb = consts.tile([P, L, 8, HID], b16, name="w_hid_sb")
    for l in range(L):
        for cc in range(8):
            dma(out=w_hid_sb[:, l, cc], in_=aps['w_hid'][l, cc])
    b_hid_sb = consts.tile([P, L, 8], f32, name="b_hid_sb")
    dma(out=b_hid_sb, in_=aps['b_hid'])
    w_out_sb = consts.tile([P, 8, D], b16, name="w_out_sb")
    for cc in range(8):
        dma(out=w_out_sb[:, cc], in_=aps['w_out'][cc])
    b_out_sb = consts.tile([P, 4], f32, name="b_out_sb")
    dma(out=b_out_sb, in_=aps['b_out'])
    wq_sb = consts.tile([P, 4, D], b16, name="wq_sb")
    wk_sb = consts.tile([P, 4, D], b16, name="wk_sb")
    wv_sb = consts.tile([P, 4, D], b16, name="wv_sb")
    wo_sb = consts.tile([P, 4, D], b16, name="wo_sb")
    for cc in range(4):
        dma(out=wq_sb[:, cc], in_=aps['wq'][cc])
        dma(out=wk_sb[:, cc], in_=aps['wk'][cc])
        dma(out=wv_sb[:, cc], in_=aps['wv'][cc])
        dma(out=wo_sb[:, cc], in_=aps['wo'][cc])
    bq_sb = consts.tile([P, 4], f32, name="bq_sb")
    dma(out=bq_sb, in_=aps['bq'])
    bk_sb = consts.tile([P, 4], f32, name="bk_sb")
    dma(out=bk_sb, in_=aps['bk'])
    bvb_sb = consts.tile([1, D], b16, name="bvb_sb")
    dma(out=bvb_sb, in_=aps['bvb'])
    bo_sb = consts.tile([P, 4], f32, name="bo_sb")
    dma(out=bo_sb, in_=aps['bo'])
    mskt_sb = consts.tile([P, 4, 3, P], b16, name="mskt_sb")
    dma(out=mskt_sb, in_=aps['msk'])

    ident_sb = consts.tile([P, P], b16, name="ident_sb")
    make_identity(nc, ident_sb)
    ones_col = consts.tile([P, 1], b16, name="ones_col")
    nc.vector.memset(ones_col, 1.0)
    ones_row = consts.tile([1, P], b16, name="ones_row")
    nc.vector.memset(ones_row, 1.0)
    ones_row_f = consts.tile([1, P], f32, name="ones_row_f")
    nc.vector.memset(ones_row_f, 1.0)
    eps_sb = consts.tile([1, 1], f32, name="eps_sb")
    nc.vector.memset(eps_sb, 1e-24)

    # persistent activations
    qlin_sb = acts.tile([P, 4, XLEN], b16, name="qlin_sb")
    rq_sb = acts.tile([P, 4, NKEY], b16, name="rq_sb")
    sq_sb = acts.tile([P, 4, NKEY], b16, name="sq_sb")
    rqn_sb = acts.tile([P, 4, NKEY], b16, name="rqn_sb")
    n_sb = acts.tile([1, NKEY], f32, name="n_sb")
    nbc_sb = acts.tile([P, NKEY], b16, name="nbc_sb")
    h_a = acts.tile([P, 8, NKEY], b16, name="h_a")
    h_b = acts.tile([P, 8, NKEY], b16, name="h_b")
    r_sb = acts.tile([P, 4, NKEY], b16, name="r_sb")
    qt_sb = acts.tile([P, 4, NQ], b16, name="qt_sb")
    kt_sb = acts.tile([P, 4, NKEY], b16, name="kt_sb")
    v2_sb = acts.tile([P, 6, NH, DH + 1], b16, name="v2_sb")
    av_tp_sb = acts.tile([P, 4, D], b16, name="av_tp_sb")
    ao_sb = acts.tile([P, 4, NQ], b16, name="ao_sb")
    nc.vector.memset(v2_sb[:, :, :, DH:DH + 1], 1.0)

    with tc.tile_pool(name="ps2", bufs=3, space="PSUM") as ps2, \
         tc.tile_pool(name="ps1", bufs=2, space="PSUM") as ps1:

        def p2(name):
            # two PSUM banks; each [:, tt, :384] matmul slice is bank-aligned
            return ps2.tile([P, 2, 512], f32, tag="ps2", name=name)

        def p1(name, shape=(P, NQ)):
            return ps1.tile(list(shape), f32, tag="ps1", name=name)

        # ---- Phase A+B interleaved per channel-chunk: q_lin then conv ----
        # q_lin tok tiles [0:512) and [512:896) so conv tt=0 only needs tile 0
        for dt_ in range(4):
            for tt, (t0, t1) in enumerate(((0, 512), (512, 896))):
                qlin_ps = p1("qlin_ps")
                for cc in range(4):
                    mm(qlin_ps[:, :t1 - t0],
                       qp_w_sb[:, cc, 128 * dt_:128 * (dt_ + 1)],
                       xt_sb[:, cc, t0:t1], start=(cc == 0), stop=(cc == 3))
                nc.vector.tensor_scalar_add(qlin_sb[:, dt_, t0:t1],
                                            qlin_ps[:, :t1 - t0],
                                            qp_b_sb[:, dt_:dt_ + 1])
            # conv for this channel chunk (key j = local token 128+j)
            cc = dt_
            rq_ps = p2("rq_ps")
            for tt in range(2):
                for k in range(K):
                    base = 125 + k + 384 * tt
                    mm(rq_ps[:, tt, :384], kdiag_sb[:, cc, k],
                       qlin_sb[:, cc, base:base + 384],
                       start=(k == 0), stop=(k == 3))
            nc.scalar.activation(rq_sb[:, cc], rq_ps[:, :, :384], AF.Copy)
            for tt in range(2):
                tsl = slice(384 * tt, 384 * (tt + 1))
                nc.vector.tensor_mul(sq_sb[:, cc, tsl], rq_sb[:, cc, tsl],
                                     rq_sb[:, cc, tsl])
        for tt in range(2):
            tsl = slice(384 * tt, 384 * (tt + 1))
            ss_ps = p1("ss_ps", (1, 384))
            for cc in range(4):
                mm(ss_ps, ones_col, sq_sb[:, cc, tsl],
                   start=(cc == 0), stop=(cc == 3))
            # sqrt(ss + 1e-24) == max(||q||, 1e-12) up to fp32 rounding; keeps
            # zero-padded halo tokens finite without a separate clamp op
            nc.scalar.activation(n_sb[:, tsl], ss_ps, AF.Sqrt, bias=eps_sb)
            nc.vector.reciprocal(n_sb[:, tsl], n_sb[:, tsl])
            nbc_ps = p1("nbc_ps", (P, 384))
            mm(nbc_ps, ones_row_f, n_sb[:, tsl], start=True, stop=True)
            nc.vector.tensor_copy(nbc_sb[:, tsl], nbc_ps)
            for cc in range(4):
                nc.vector.tensor_mul(rqn_sb[:, cc, tsl], rq_sb[:, cc, tsl],
                                     nbc_sb[:, tsl])

        # ---- Phase D: ResLinear MLP (768-wide point ops) ----
        for j in range(8):
            mi_ps = p2("mi_ps")
            for tt in range(2):
                tsl = slice(384 * tt, 384 * (tt + 1))
                for cc in range(4):
                    mm(mi_ps[:, tt, :384], w_in_sb[:, cc, 128 * j:128 * (j + 1)],
                       rqn_sb[:, cc, tsl], start=(cc == 0), stop=(cc == 3))
            nc.scalar.activation(h_a[:, j], mi_ps[:, :, :384],
                                 AF.Silu, bias=b_in_sb[:, j:j + 1])
        for l in range(L):
            src, dst = (h_a, h_b) if l == 0 else (h_b, h_a)
            for j in range(8):
                mh_ps = p2("mh_ps")
                for tt in range(2):
                    tsl = slice(384 * tt, 384 * (tt + 1))
                    for cc in range(8):
                        mm(mh_ps[:, tt, :384],
                           w_hid_sb[:, l, cc, 128 * j:128 * (j + 1)],
                           src[:, cc, tsl], start=(cc == 0), stop=(cc == 7))
                slu = work.tile([P, NKEY], b16, name="slu")
                nc.scalar.activation(slu, mh_ps[:, :, :384],
                                     AF.Silu, bias=b_hid_sb[:, l, j:j + 1])
                nc.vector.tensor_add(dst[:, j], src[:, j], slu)
        h_fin = h_a
        for dt_ in range(4):
            mo_ps = p2("mo_ps")
            for tt in range(2):
                tsl = slice(384 * tt, 384 * (tt + 1))
                for cc in range(8):
                    mm(mo_ps[:, tt, :384], w_out_sb[:, cc, 128 * dt_:128 * (dt_ + 1)],
                       h_fin[:, cc, tsl], start=(cc == 0), stop=(cc == 7))
            nc.scalar.activation(r_sb[:, dt_], mo_ps[:, :, :384],
                                 AF.Silu, bias=b_out_sb[:, dt_:dt_ + 1])

        # ---- Phase E: QKV projections ----
        for dt_ in range(4):
            q_ps = p1("q_ps")
            for cc in range(4):
                mm(q_ps, wq_sb[:, cc, 128 * dt_:128 * (dt_ + 1)],
                   r_sb[:, cc, 256:768], start=(cc == 0), stop=(cc == 3))
            nc.vector.tensor_scalar_add(qt_sb[:, dt_], q_ps, bq_sb[:, dt_:dt_ + 1])
            k_ps = p2("k_ps")
            for tt in range(2):
                tsl = slice(384 * tt, 384 * (tt + 1))
                for cc in range(4):
                    mm(k_ps[:, tt, :384], wk_sb[:, cc, 128 * dt_:128 * (dt_ + 1)],
                       r_sb[:, cc, tsl], start=(cc == 0), stop=(cc == 3))
            nc.vector.tensor_scalar_add(kt_sb[:, dt_], k_ps[:, :, :384],
                                        bk_sb[:, dt_:dt_ + 1])
        for tb in range(6):
            v_ps = p1("v_ps")
            mm(v_ps, ones_row, bvb_sb, start=True, stop=False)  # rank-1 bias
            for cc in range(4):
                mm(v_ps, r_sb[:, cc, 128 * tb:128 * (tb + 1)], wv_sb[:, cc],
                   start=False, stop=(cc == 3))
            nc.vector.tensor_copy(v2_sb[:, tb, :, :DH],
                                  v_ps.rearrange("p (h d) -> p h d", h=NH))

    # ---- Phase F: attention with transposed scores (k on partitions) ----
    # sT[k, q] = K_h Q_h^T + mask^T; p = exp(sT/8); AV uses p-chunks as lhsT so
    # the output lands token-partitioned, with a 65th ones-column of V giving
    # the softmax denominator for free. Normalization happens on the AV output
    # (per-partition scalars), then a final PE transpose back to CP layout.
    with tc.tile_pool(name="psc", bufs=4, space="PSUM") as psc, \
         tc.tile_pool(name="psav", bufs=2, space="PSUM") as psav, \
         tc.tile_pool(name="pstr", bufs=2, space="PSUM") as pstr:
        for qb in range(4):

            def emit_scores(hp, qb=qb):
                tiles = []
                for i in (0, 1):
                    pbase = 64 * i
                    st_ps = psc.tile([P, 3, P], DT.float32, tag="sc", name="st_ps")
                    for kc in range(3):
                        # kc==1 covers keys [q-127, q+128) minus edges: fully
                        # inside the causal window for every q; only qb==0 can
                        # clip it at the sequence start (kmin).
                        need_mask = (kc != 1) or (qb == 0)
                        if need_mask:
                            mm(st_ps[:, kc, :], ident_sb, mskt_sb[:, qb, kc],
                               start=True, stop=False)
                        mm(st_ps[:, kc, :],
                           kt_sb[pbase:pbase + 64, hp, 128 * (qb + kc):128 * (qb + kc + 1)],
                           qt_sb[pbase:pbase + 64, hp, 128 * qb:128 * (qb + 1)],
                           start=(not need_mask), stop=True)
                    tiles.append(st_ps)
                return tiles

            def emit_softmax_av(hp, tiles, qb=qb):
                for i in (0, 1):
                    h = 2 * hp + i
                    pts_sb = work.tile([P, 384], DT.bfloat16, name="pts_sb")
                    nc.scalar.activation(pts_sb, tiles[i], AF.Exp, scale=0.125)
                    av_ps = psav.tile([P, DH + 1], DT.float32, tag="av", name="av_ps")
                    for kc in range(3):
                        mm(av_ps, pts_sb[:, 128 * kc:128 * (kc + 1)],
                           v2_sb[:, qb + kc, h, :],
                           start=(kc == 0), stop=(kc == 2))
                    rs = work.tile([P, 1], DT.float32, name="rs")
                    nc.vector.reciprocal(rs, av_ps[:, DH:DH + 1])
                    nc.vector.tensor_scalar_mul(av_tp_sb[:, qb, DH * h:DH * (h + 1)],
                                                av_ps[:, :DH], rs)

            pend = None
            for hp in range(4):
                tiles = emit_scores(hp)
                if pend is not None:
                    emit_softmax_av(*pend)
                pend = (hp, tiles)
            emit_softmax_av(*pend)
            # transpose attention output back to CP layout for the o-proj
            tr_ps = pstr.tile([P, 4, P], DT.float32, tag="tr", name="tr_ps")
            for cc in range(4):
                mm(tr_ps[:, cc, :], av_tp_sb[:, qb, 128 * cc:128 * (cc + 1)],
                   ident_sb, start=True, stop=True)
            nc.vector.tensor_copy(ao_sb[:, :, 128 * qb:128 * (qb + 1)], tr_ps)

    # ---- Phase G: output projection ----
    with tc.tile_pool(name="ps3", bufs=2, space="PSUM") as ps3:
        for dt_ in range(4):
            o_ps = ps3.tile([P, NQ], DT.float32, tag="o", name="o_ps")
            for cc in range(4):
                mm(o_ps, wo_sb[:, cc, 128 * dt_:128 * (dt_ + 1)], ao_sb[:, cc],
                   start=(cc == 0), stop=(cc == 3))
            o_sb = work.tile([P, NQ], DT.float32, name="o_sb", bufs=2)
            nc.vector.tensor_scalar_add(o_sb, o_ps, bo_sb[:, dt_:dt_ + 1])
            dma(out=aps['out'][dt_], in_=o_sb)


def _host_prep(inputs):
    f32 = np.float32
    x = inputs['x'].astype(f32)
    mmem = np.broadcast_to(inputs['meta_memory'].astype(f32)[None], (B, M, D))
    xm = np.concatenate([mmem, x], axis=1)  # [B, T, D]

    def chunkw(w, p=P):  # [n, m] -> [n//p, p, m]
        return np.ascontiguousarray(w.reshape(-1, p, w.shape[-1]))

    kern = inputs['qp_conv'].astype(f32)  # [K, D]
    kd = np.zeros((4, K, P, P), f32)
    for cc in range(4):
        for k in range(K):
            np.fill_diagonal(kd[cc, k], kern[k, 128 * cc:128 * (cc + 1)])

    shared = {
        'qp_w': chunkw(inputs['qp_w'].astype(bf16)),
        'kdiag': kd.astype(bf16),
        'qp_b': np.ascontiguousarray(inputs['qp_b'].astype(f32).reshape(4, P).T),
        'w_in': chunkw(inputs['mlp_w_in'].astype(bf16)),
        'b_in': np.ascontiguousarray(inputs['mlp_b_in'].astype(f32).reshape(8, P).T),
        'w_hid': np.ascontiguousarray(
            inputs['mlp_w_hid'].astype(bf16).reshape(L, 8, P, HID)),
        'b_hid': np.ascontiguousarray(
            inputs['mlp_b_hid'].astype(f32).reshape(L, 8, P).transpose(2, 0, 1)),
        'w_out': chunkw(inputs['mlp_w_out'].astype(bf16)),
        'b_out': np.ascontiguousarray(inputs['mlp_b_out'].astype(f32).reshape(4, P).T),
        'wq': chunkw(inputs['swa_wq'].astype(bf16)),
        'bq': np.ascontiguousarray(inputs['swa_bq'].astype(f32).reshape(4, P).T),
        'wk': chunkw(inputs['swa_wk'].astype(bf16)),
        'bk': np.ascontiguousarray(inputs['swa_bk'].astype(f32).reshape(4, P).T),
        'wv': chunkw(inputs['swa_wv'].astype(bf16)),
        'bvb': np.ascontiguousarray(inputs['swa_bv'].astype(bf16)[None, :]),
        'wo': chunkw(inputs['swa_wo'].astype(bf16)),
        'bo': np.ascontiguousarray(inputs['swa_bo'].astype(f32).reshape(4, P).T),
    }

    pp = np.arange(P)[:, None]
    cc = np.arange(384)[None, :]
    band = (cc - pp >= 1) & (cc - pp <= 256)
    in_maps = []
    for core in range(NCORES):
        b, c = divmod(core, 4)
        qg0 = M + c * CHUNK
        Xl = np.zeros((XLEN, D), f32)
        g0 = qg0 - HALO
        s0 = max(0, g0)
        Xl[s0 - g0:] = xm[b, s0:qg0 + CHUNK]
        xt = np.ascontiguousarray(Xl.T.reshape(4, P, XLEN)).astype(bf16)
        kmin = 192 if c == 0 else 0
        m = np.empty((4, P, 384), f32)
        for qb in range(4):
            allowed = band & (qb * P + cc >= kmin)
            m[qb] = np.where(allowed, 0.0, -8e9)
        # transpose to [k-within-chunk, qb, kc, q]
        mt = m.reshape(4, P, 3, P).transpose(3, 0, 2, 1)
        msk = np.ascontiguousarray(mt).astype(bf16)
        im = dict(shared)
        im['xt'] = xt
        im['msk'] = msk
        in_maps.append(im)
    return in_maps


_NC_CACHE = None
_HOOK_WRAPPED = False


def _wrap_cc_hook():
    """Surface the real exception when the PJRT python compile hook fails
    (otherwise it shows as an opaque CallFunctionObjArgs error)."""
    global _HOOK_WRAPPED
    if _HOOK_WRAPPED:
        return
    try:
        from concourse import bass2jax
        import libneuronxla
        import traceback
        import sys
        bass2jax.install_neuronx_cc_hook()
        orig = libneuronxla.neuronx_cc

        def wrapped(*a, **k):
            try:
                return orig(*a, **k)
            except Exception:
                traceback.print_exc(file=sys.stderr)
                sys.stderr.flush()
                raise

        libneuronxla.neuronx_cc = wrapped
        bass2jax.install_neuronx_cc_hook = lambda: None
        _HOOK_WRAPPED = True
    except ImportError:
        pass


def kernel(**inputs):
    _wrap_cc_hook()
    global _NC_CACHE, LAST_RESULTS
    inputs = {k: np.asarray(v) for k, v in inputs.items()}
    in_maps = _host_prep(inputs)
    if _NC_CACHE is None:
        _NC_CACHE = _build_nc()
    nc = _NC_CACHE
    res = run_bass_kernel_spmd(nc, in_maps, core_ids=list(range(NCORES)))
    LAST_RESULTS = res
    outf = np.zeros((B, S, D), np.float32)
    for core in range(NCORES):
        b, c = divmod(core, 4)
        o_cp = res.results[core]['out'].reshape(D, NQ)
        outf[b, c * CHUNK:(c + 1) * CHUNK, :] = o_cp.T
    return outf


# revision 19
# speedup vs baseline: 1.0118x; 1.0118x over previous
"""Trainium2 Bass kernel for nn_NeuralMemory (meta-memory injection + query conv
projection + l2norm + ResLinear MLP + sliding-window attention).

Strategy: shard the 2*2048 output tokens over 8 cores as (batch b, chunk c) with
c covering 512 query tokens. Each core recomputes the MLP for its 512 owned
tokens plus a 256-token left halo (the attention key window) -- no collectives.
All compute stays in "CP" layout (channels on partitions, tokens on free dim),
so every matmul uses the weights as lhsT exactly as stored and activations as
rhs; V is produced in token-partition layout by using the activations as lhsT.

v2: conv as PE diagonal matmuls, tt-pipelined l2norm, 768-wide MLP point ops on
2-bank PSUM tiles, head-pair software-pipelined attention, scoped PSUM pools.
"""
import os
import numpy as np
import ml_dtypes
from contextlib import ExitStack

import concourse.bass as bass
import concourse.tile as tile
from concourse import bacc, mybir
from concourse.bass_utils import run_bass_kernel_spmd
from concourse.masks import make_identity

AF = mybir.ActivationFunctionType
ALU = mybir.AluOpType
DT = mybir.dt
bf16 = ml_dtypes.bfloat16

B, S, D, HID, M = 2, 2048, 512, 1024, 64
K, NH, DH, WIN, L = 4, 8, 64, 256, 2
T = M + S                      # 2112
CHUNK = 512                    # owned query tokens per core
HALO = 384                     # left halo in X-token space (keys start at local 128)
XLEN = HALO + CHUNK            # 896
NKEY = 768                     # key positions per core (local tokens 128..896)
NQ = 512
P = 128
NCORES = 8

LAST_RESULTS = None


def _build_nc(repeat=1):
    nc = bacc.Bacc("TRN2", target_bir_lowering=False, debug=False,
                   enable_asserts=False)

    def din(name, shape, dt):
        return nc.dram_tensor(name, shape, dt, kind="ExternalInput").ap()

    xt = din("xt", (4, P, XLEN), DT.bfloat16)
    msk = din("msk", (P, 4, 3, P), DT.bfloat16)
    qp_w = din("qp_w", (4, P, D), DT.bfloat16)
    kdiag = din("kdiag", (4, K, P, P), DT.bfloat16)
    qp_b = din("qp_b", (P, 4), DT.float32)
    w_in = din("w_in", (4, P, HID), DT.bfloat16)
    b_in = din("b_in", (P, 8), DT.float32)
    w_hid = din("w_hid", (L, 8, P, HID), DT.bfloat16)
    b_hid = din("b_hid", (P, L, 8), DT.float32)
    w_out = din("w_out", (8, P, D), DT.bfloat16)
    b_out = din("b_out", (P, 4), DT.float32)
    wq = din("wq", (4, P, D), DT.bfloat16)
    bq = din("bq", (P, 4), DT.float32)
    wk = din("wk", (4, P, D), DT.bfloat16)
    bk = din("bk", (P, 4), DT.float32)
    wv = din("wv", (4, P, D), DT.bfloat16)
    bvb = din("bvb", (1, D), DT.bfloat16)
    wo = din("wo", (4, P, D), DT.bfloat16)
    bo = din("bo", (P, 4), DT.float32)
    out = nc.dram_tensor("out", (4, P, NQ), DT.float32, kind="ExternalOutput").ap()

    aps = {k: v for k, v in locals().items() if k not in ('nc', 'din')}
    with tile.TileContext(nc) as tc:
        for _ in range(repeat):
            with ExitStack() as ctx:
                _body(ctx, tc, aps)
    nc.finalize()
    return nc


def _body(ctx, tc, aps):
    nc = tc.nc
    f32 = DT.float32
    b16 = DT.bfloat16

    consts = ctx.enter_context(tc.tile_pool(name="consts", bufs=1))
    acts = ctx.enter_context(tc.tile_pool(name="acts", bufs=1))
    work = ctx.enter_context(tc.tile_pool(name="work", bufs=4))

    dma = nc.sync.dma_start
    mm = nc.tensor.matmul

    # ---- stream inputs into SBUF ----
    xt_sb = acts.tile([P, 4, XLEN], b16, name="xt_sb")
    qp_w_sb = consts.tile([P, 4, D], b16, name="qp_w_sb")
    for cc in range(4):
        dma(out=xt_sb[:, cc], in_=aps['xt'][cc])
        dma(out=qp_w_sb[:, cc], in_=aps['qp_w'][cc])
    kdiag_sb = consts.tile([P, 4, K, P], b16, name="kdiag_sb")
    for cc in range(4):
        dma(out=kdiag_sb[:, cc], in_=aps['kdiag'][cc].rearrange("k p n -> p k n"))
    qp_b_sb = consts.tile([P, 4], f32, name="qp_b_sb")
    dma(out=qp_b_sb, in_=aps['qp_b'])
    w_in_sb = consts.tile([P, 4, HID], b16, name="w_in_sb")
    for cc in range(4):
        dma(out=w_in_sb[:, cc], in_=aps['w_in'][cc])
    b_in_sb = consts.tile([P, 8], f32, name="b_in_sb")
    dma(out=b_in_sb, in_=aps['b_in'])
    w_hid_sb = consts.tile([P, L, 8, HID], b16, name="w_hid_sb")
    for l in range(L):
        for cc in range(8):
            dma(out=w_hid_sb[:, l, cc], in_=aps['w_hid'][l, cc])
    b_hid_sb = consts.tile([P, L, 8], f32, name="b_hid_sb")
    dma(out=b_hid_sb, in_=aps['b_hid'])
    w_out_sb = consts.tile([P, 8, D], b16, name="w_out_sb")
    for cc in range(8):
        dma(out=w_out_sb[:, cc], in_=aps['w_out'][cc])
    b_out_sb = consts.tile([P, 4], f32, name="b_out_sb")
    dma(out=b_out_sb, in_=aps['b_out'])
    wq_sb = consts.tile([P, 4, D], b16, name="wq_sb")
    wk_sb = consts.tile([P, 4, D], b16, name="wk_sb")
    wv_sb = consts.tile([P, 4, D], b16, name="wv_sb")
    wo_sb = consts.tile([P, 4, D], b16, name="wo_sb")
    for cc in range(4):
        dma(out=wq_sb[:, cc], in_=aps['wq'][cc])
        dma(out=wk_sb[:, cc], in_=aps['wk'][cc])
        dma(out=wv_sb[:, cc], in_=aps['wv'][cc])
        dma(out=wo_sb[:, cc], in_=aps['wo'][cc])
    bq_sb = consts.tile([P, 4], f32, name="bq_sb")
    dma(out=bq_sb, in_=aps['bq'])
    bk_sb = consts.tile([P, 4], f32, name="bk_sb")
    dma(out=bk_sb, in_=aps['bk'])
    bvb_sb = consts.tile([1, D], b16, name="bvb_sb")
    dma(out=bvb_sb, in_=aps['bvb'])
    bo_sb = consts.tile([P, 4], f32, name="bo_sb")
    dma(out=bo_sb, in_=aps['bo'])
    mskt_sb = consts.tile([P, 4, 3, P], b16, name="mskt_sb")
    dma(out=mskt_sb, in_=aps['msk'])

    ident_sb = consts.tile([P, P], b16, name="ident_sb")
    make_identity(nc, ident_sb)
    ones_col = consts.tile([P, 1], b16, name="ones_col")
    nc.vector.memset(ones_col, 1.0)
    ones_row = consts.tile([1, P], b16, name="ones_row")
    nc.vector.memset(ones_row, 1.0)
    ones_row_f = consts.tile([1, P], f32, name="ones_row_f")
    nc.vector.memset(ones_row_f, 1.0)
    eps_sb = consts.tile([1, 1], f32, name="eps_sb")
    nc.vector.memset(eps_sb, 1e-24)

    # persistent activations
    qlin_sb = acts.tile([P, 4, XLEN], b16, name="qlin_sb")
    rq_sb = acts.tile([P, 4, NKEY], b16, name="rq_sb")
    sq_sb = acts.tile([P, 4, NKEY], b16, name="sq_sb")
    rqn_sb = acts.tile([P, 4, NKEY], b16, name="rqn_sb")
    n_sb = acts.tile([1, NKEY], f32, name="n_sb")
    nbc_sb = acts.tile([P, NKEY], b16, name="nbc_sb")
    h_a = acts.tile([P, 8, NKEY], b16, name="h_a")
    h_b = acts.tile([P, 8, NKEY], b16, name="h_b")
    r_sb = acts.tile([P, 4, NKEY], b16, name="r_sb")
    qt_sb = acts.tile([P, 4, NQ], b16, name="qt_sb")
    kt_sb = acts.tile([P, 4, NKEY], b16, name="kt_sb")
    v2_sb = acts.tile([P, 6, NH, DH + 1], b16, name="v2_sb")
    av_tp_sb = acts.tile([P, 4, D], b16, name="av_tp_sb")
    ao_sb = acts.tile([P, 4, NQ], b16, name="ao_sb")
    nc.vector.memset(v2_sb[:, :, :, DH:DH + 1], 1.0)

    with tc.tile_pool(name="ps2", bufs=3, space="PSUM") as ps2, \
         tc.tile_pool(name="ps1", bufs=2, space="PSUM") as ps1:

        def p2(name):
            # two PSUM banks; each [:, tt, :384] matmul slice is bank-aligned
            return ps2.tile([P, 2, 512], f32, tag="ps2", name=name)

        def p1(name, shape=(P, NQ)):
            return ps1.tile(list(shape), f32, tag="ps1", name=name)

        # ---- Phase A+B interleaved per channel-chunk: q_lin then conv ----
        # q_lin only needed for tokens [125:896) (conv taps); tile split at 512
        # so conv tt=0 depends only on the first tile.
        for dt_ in range(4):
            for tt, (t0, t1) in enumerate(((125, 512), (512, 896))):
                qlin_ps = p1("qlin_ps")
                for cc in range(4):
                    mm(qlin_ps[:, :t1 - t0],
                       qp_w_sb[:, cc, 128 * dt_:128 * (dt_ + 1)],
                       xt_sb[:, cc, t0:t1], start=(cc == 0), stop=(cc == 3))
                nc.vector.tensor_scalar_add(qlin_sb[:, dt_, t0:t1],
                                            qlin_ps[:, :t1 - t0],
                                            qp_b_sb[:, dt_:dt_ + 1])
            # conv for this channel chunk (key j = local token 128+j); copy and
            # square per token-tile so the norm chain starts during tt=1 conv
            cc = dt_
            rq_ps = p2("rq_ps")
            for tt in range(2):
                for k in range(K):
                    base = 125 + k + 384 * tt
                    mm(rq_ps[:, tt, :384], kdiag_sb[:, cc, k],
                       qlin_sb[:, cc, base:base + 384],
                       start=(k == 0), stop=(k == 3))
            nc.scalar.activation(rq_sb[:, cc], rq_ps[:, :, :384], AF.Copy)
            for tt in range(2):
                tsl = slice(384 * tt, 384 * (tt + 1))
                nc.vector.tensor_mul(sq_sb[:, cc, tsl], rq_sb[:, cc, tsl],
                                     rq_sb[:, cc, tsl])
        for tt in range(2):
            tsl = slice(384 * tt, 384 * (tt + 1))
            ss_ps = p1("ss_ps", (1, 384))
            for cc in range(4):
                mm(ss_ps, ones_col, sq_sb[:, cc, tsl],
                   start=(cc == 0), stop=(cc == 3))
            # sqrt(ss + 1e-24) == max(||q||, 1e-12) up to fp32 rounding; keeps
            # zero-padded halo tokens finite without a separate clamp op
            nc.scalar.activation(n_sb[:, tsl], ss_ps, AF.Sqrt, bias=eps_sb)
            nc.vector.reciprocal(n_sb[:, tsl], n_sb[:, tsl])
            nbc_ps = p1("nbc_ps", (P, 384))
            mm(nbc_ps, ones_row_f, n_sb[:, tsl], start=True, stop=True)
            nc.vector.tensor_copy(nbc_sb[:, tsl], nbc_ps)
            for cc in range(4):
                nc.vector.tensor_mul(rqn_sb[:, cc, tsl], rq_sb[:, cc, tsl],
                                     nbc_sb[:, tsl])

        # ---- Phase D: ResLinear MLP (768-wide point ops) ----
        for j in range(8):
            mi_ps = p2("mi_ps")
            for tt in range(2):
                tsl = slice(384 * tt, 384 * (tt + 1))
                for cc in range(4):
                    mm(mi_ps[:, tt, :384], w_in_sb[:, cc, 128 * j:128 * (j + 1)],
                       rqn_sb[:, cc, tsl], start=(cc == 0), stop=(cc == 3))
            nc.scalar.activation(h_a[:, j], mi_ps[:, :, :384],
                                 AF.Silu, bias=b_in_sb[:, j:j + 1])
        for l in range(L):
            src, dst = (h_a, h_b) if l == 0 else (h_b, h_a)
            for j in range(8):
                mh_ps = p2("mh_ps")
                for tt in range(2):
                    tsl = slice(384 * tt, 384 * (tt + 1))
                    for cc in range(8):
                        mm(mh_ps[:, tt, :384],
                           w_hid_sb[:, l, cc, 128 * j:128 * (j + 1)],
                           src[:, cc, tsl], start=(cc == 0), stop=(cc == 7))
                slu = work.tile([P, NKEY], b16, name="slu")
                nc.scalar.activation(slu, mh_ps[:, :, :384],
                                     AF.Silu, bias=b_hid_sb[:, l, j:j + 1])
                nc.vector.tensor_add(dst[:, j], src[:, j], slu)
        h_fin = h_a
        for dt_ in range(4):
            mo_ps = p2("mo_ps")
            for tt in range(2):
                tsl = slice(384 * tt, 384 * (tt + 1))
                for cc in range(8):
                    mm(mo_ps[:, tt, :384], w_out_sb[:, cc, 128 * dt_:128 * (dt_ + 1)],
                       h_fin[:, cc, tsl], start=(cc == 0), stop=(cc == 7))
            nc.scalar.activation(r_sb[:, dt_], mo_ps[:, :, :384],
                                 AF.Silu, bias=b_out_sb[:, dt_:dt_ + 1])

        # ---- Phase E: QKV projections ----
        for dt_ in range(4):
            q_ps = p1("q_ps")
            for cc in range(4):
                mm(q_ps, wq_sb[:, cc, 128 * dt_:128 * (dt_ + 1)],
                   r_sb[:, cc, 256:768], start=(cc == 0), stop=(cc == 3))
            nc.vector.tensor_scalar_add(qt_sb[:, dt_], q_ps, bq_sb[:, dt_:dt_ + 1])
            k_ps = p2("k_ps")
            for tt in range(2):
                tsl = slice(384 * tt, 384 * (tt + 1))
                for cc in range(4):
                    mm(k_ps[:, tt, :384], wk_sb[:, cc, 128 * dt_:128 * (dt_ + 1)],
                       r_sb[:, cc, tsl], start=(cc == 0), stop=(cc == 3))
            nc.vector.tensor_scalar_add(kt_sb[:, dt_], k_ps[:, :, :384],
                                        bk_sb[:, dt_:dt_ + 1])
        for tb in range(6):
            v_ps = p1("v_ps")
            mm(v_ps, ones_row, bvb_sb, start=True, stop=False)  # rank-1 bias
            for cc in range(4):
                mm(v_ps, r_sb[:, cc, 128 * tb:128 * (tb + 1)], wv_sb[:, cc],
                   start=False, stop=(cc == 3))
            nc.vector.tensor_copy(v2_sb[:, tb, :, :DH],
                                  v_ps.rearrange("p (h d) -> p h d", h=NH))

    # ---- Phase F: attention with transposed scores (k on partitions) ----
    # sT[k, q] = K_h Q_h^T + mask^T; p = exp(sT/8); AV uses p-chunks as lhsT so
    # the output lands token-partitioned, with a 65th ones-column of V giving
    # the softmax denominator for free. Normalization happens on the AV output
    # (per-partition scalars), then a final PE transpose back to CP layout.
    with tc.tile_pool(name="psc", bufs=4, space="PSUM") as psc, \
         tc.tile_pool(name="psav", bufs=2, space="PSUM") as psav, \
         tc.tile_pool(name="pstr", bufs=2, space="PSUM") as pstr:
        for qb in range(4):

            def emit_scores(hp, qb=qb):
                tiles = []
                for i in (0, 1):
                    pbase = 64 * i
                    st_ps = psc.tile([P, 3, P], DT.float32, tag="sc", name="st_ps")
                    for kc in range(3):
                        # kc==1 covers keys [q-127, q+128) minus edges: fully
                        # inside the causal window for every q; only qb==0 can
                        # clip it at the sequence start (kmin).
                        need_mask = (kc != 1) or (qb == 0)
                        if need_mask:
                            mm(st_ps[:, kc, :], ident_sb, mskt_sb[:, qb, kc],
                               start=True, stop=False)
                        mm(st_ps[:, kc, :],
                           kt_sb[pbase:pbase + 64, hp, 128 * (qb + kc):128 * (qb + kc + 1)],
                           qt_sb[pbase:pbase + 64, hp, 128 * qb:128 * (qb + 1)],
                           start=(not need_mask), stop=True)
                    tiles.append(st_ps)
                return tiles

            def emit_softmax_av(hp, tiles, qb=qb):
                for i in (0, 1):
                    h = 2 * hp + i
                    pts_sb = work.tile([P, 384], DT.bfloat16, name="pts_sb")
                    nc.scalar.activation(pts_sb, tiles[i], AF.Exp, scale=0.125)
                    av_ps = psav.tile([P, DH + 1], DT.float32, tag="av", name="av_ps")
                    for kc in range(3):
                        mm(av_ps, pts_sb[:, 128 * kc:128 * (kc + 1)],
                           v2_sb[:, qb + kc, h, :],
                           start=(kc == 0), stop=(kc == 2))
                    rs = work.tile([P, 1], DT.float32, name="rs")
                    nc.vector.reciprocal(rs, av_ps[:, DH:DH + 1])
                    nc.vector.tensor_scalar_mul(av_tp_sb[:, qb, DH * h:DH * (h + 1)],
                                                av_ps[:, :DH], rs)

            pend = None
            for hp in range(4):
                tiles = emit_scores(hp)
                if pend is not None:
                    emit_softmax_av(*pend)
                pend = (hp, tiles)
            emit_softmax_av(*pend)
            # transpose attention output back to CP layout for the o-proj
            tr_ps = pstr.tile([P, 4, P], DT.float32, tag="tr", name="tr_ps")
            for cc in range(4):
                mm(tr_ps[:, cc, :], av_tp_sb[:, qb, 128 * cc:128 * (cc + 1)],
                   ident_sb, start=True, stop=True)
            nc.vector.tensor_copy(ao_sb[:, :, 128 * qb:128 * (qb + 1)], tr_ps)

    # ---- Phase G: output projection ----
    with tc.tile_pool(name="ps3", bufs=2, space="PSUM") as ps3:
        for dt_ in range(4):
            o_ps = ps3.tile([P, NQ], DT.float32, tag="o", name="o_ps")
            for cc in range(4):
                mm(o_ps, wo_sb[:, cc, 128 * dt_:128 * (dt_ + 1)], ao_sb[:, cc],
                   start=(cc == 0), stop=(cc == 3))
            o_sb = work.tile([P, NQ], DT.float32, name="o_sb", bufs=2)
            nc.vector.tensor_scalar_add(o_sb, o_ps, bo_sb[:, dt_:dt_ + 1])
            dma(out=aps['out'][dt_], in_=o_sb)


def _host_prep(inputs):
    f32 = np.float32
    x = inputs['x'].astype(f32)
    mmem = np.broadcast_to(inputs['meta_memory'].astype(f32)[None], (B, M, D))
    xm = np.concatenate([mmem, x], axis=1)  # [B, T, D]

    def chunkw(w, p=P):  # [n, m] -> [n//p, p, m]
        return np.ascontiguousarray(w.reshape(-1, p, w.shape[-1]))

    kern = inputs['qp_conv'].astype(f32)  # [K, D]
    kd = np.zeros((4, K, P, P), f32)
    for cc in range(4):
        for k in range(K):
            np.fill_diagonal(kd[cc, k], kern[k, 128 * cc:128 * (cc + 1)])

    shared = {
        'qp_w': chunkw(inputs['qp_w'].astype(bf16)),
        'kdiag': kd.astype(bf16),
        'qp_b': np.ascontiguousarray(inputs['qp_b'].astype(f32).reshape(4, P).T),
        'w_in': chunkw(inputs['mlp_w_in'].astype(bf16)),
        'b_in': np.ascontiguousarray(inputs['mlp_b_in'].astype(f32).reshape(8, P).T),
        'w_hid': np.ascontiguousarray(
            inputs['mlp_w_hid'].astype(bf16).reshape(L, 8, P, HID)),
        'b_hid': np.ascontiguousarray(
            inputs['mlp_b_hid'].astype(f32).reshape(L, 8, P).transpose(2, 0, 1)),
        'w_out': chunkw(inputs['mlp_w_out'].astype(bf16)),
        'b_out': np.ascontiguousarray(inputs['mlp_b_out'].astype(f32).reshape(4, P).T),
        'wq': chunkw(inputs['swa_wq'].astype(bf16)),
        'bq': np.ascontiguousarray(inputs['swa_bq'].astype(f32).reshape(4, P).T),
        'wk': chunkw(inputs['swa_wk'].astype(bf16)),
        'bk': np.ascontiguousarray(inputs['swa_bk'].astype(f32).reshape(4, P).T),
        'wv': chunkw(inputs['swa_wv'].astype(bf16)),
        'bvb': np.ascontiguousarray(inputs['swa_bv'].astype(bf16)[None, :]),
        'wo': chunkw(inputs['swa_wo'].astype(bf16)),
        'bo': np.ascontiguousarray(inputs['swa_bo'].astype(f32).reshape(4, P).T),
    }

    pp = np.arange(P)[:, None]
    cc = np.arange(384)[None, :]
    band = (cc - pp >= 1) & (cc - pp <= 256)
    in_maps = []
    for core in range(NCORES):
        b, c = divmod(core, 4)
        qg0 = M + c * CHUNK
        Xl = np.zeros((XLEN, D), f32)
        g0 = qg0 - HALO
        s0 = max(0, g0)
        Xl[s0 - g0:] = xm[b, s0:qg0 + CHUNK]
        xt = np.ascontiguousarray(Xl.T.reshape(4, P, XLEN)).astype(bf16)
        kmin = 192 if c == 0 else 0
        m = np.empty((4, P, 384), f32)
        for qb in range(4):
            allowed = band & (qb * P + cc >= kmin)
            m[qb] = np.where(allowed, 0.0, -8e9)
        # transpose to [k-within-chunk, qb, kc, q]
        mt = m.reshape(4, P, 3, P).transpose(3, 0, 2, 1)
        msk = np.ascontiguousarray(mt).astype(bf16)
        im = dict(shared)
        im['xt'] = xt
        im['msk'] = msk
        in_maps.append(im)
    return in_maps


_NC_CACHE = None
_HOOK_WRAPPED = False


def _wrap_cc_hook():
    """Surface the real exception when the PJRT python compile hook fails
    (otherwise it shows as an opaque CallFunctionObjArgs error)."""
    global _HOOK_WRAPPED
    if _HOOK_WRAPPED:
        return
    try:
        from concourse import bass2jax
        import libneuronxla
        import traceback
        import sys
        bass2jax.install_neuronx_cc_hook()
        orig = libneuronxla.neuronx_cc

        def wrapped(*a, **k):
            try:
                return orig(*a, **k)
            except Exception:
                traceback.print_exc(file=sys.stderr)
                sys.stderr.flush()
                raise

        libneuronxla.neuronx_cc = wrapped
        bass2jax.install_neuronx_cc_hook = lambda: None
        _HOOK_WRAPPED = True
    except ImportError:
        pass


def kernel(**inputs):
    _wrap_cc_hook()
    global _NC_CACHE, LAST_RESULTS
    inputs = {k: np.asarray(v) for k, v in inputs.items()}
    in_maps = _host_prep(inputs)
    if _NC_CACHE is None:
        _NC_CACHE = _build_nc()
    nc = _NC_CACHE
    res = run_bass_kernel_spmd(nc, in_maps, core_ids=list(range(NCORES)))
    LAST_RESULTS = res
    outf = np.zeros((B, S, D), np.float32)
    for core in range(NCORES):
        b, c = divmod(core, 4)
        o_cp = res.results[core]['out'].reshape(D, NQ)
        outf[b, c * CHUNK:(c + 1) * CHUNK, :] = o_cp.T
    return outf


# revision 21
# speedup vs baseline: 1.0350x; 1.0229x over previous
"""Trainium2 Bass kernel for nn_NeuralMemory (meta-memory injection + query conv
projection + l2norm + ResLinear MLP + sliding-window attention).

Strategy: shard the 2*2048 output tokens over 8 cores as (batch b, chunk c) with
c covering 512 query tokens. Each core recomputes the MLP for its 512 owned
tokens plus a 256-token left halo (the attention key window) -- no collectives.
All compute stays in "CP" layout (channels on partitions, tokens on free dim),
so every matmul uses the weights as lhsT exactly as stored and activations as
rhs; V is produced in token-partition layout by using the activations as lhsT.

Perf structure (TimelineSim ~134 us/core, PE ~107 us busy): conv as PE diagonal
matmuls interleaved with q_lin per channel-chunk; tt-pipelined l2norm; 768-wide
MLP point ops on 2-bank PSUM tiles; attention with transposed scores (keys on
partitions) so exp output feeds AV directly and a ones-column of V yields the
softmax denominator in the same matmul; head-pair software pipelining; scoped
PSUM pools. PSUM matmul slices are kept bank-aligned and accumulation groups
are never interleaved across tiles (both corrupt results otherwise).
"""
import os
import numpy as np
import ml_dtypes
from contextlib import ExitStack

import concourse.bass as bass
import concourse.tile as tile
from concourse import bacc, mybir
from concourse.bass_utils import run_bass_kernel_spmd
from concourse.masks import make_identity

AF = mybir.ActivationFunctionType
ALU = mybir.AluOpType
DT = mybir.dt
bf16 = ml_dtypes.bfloat16

B, S, D, HID, M = 2, 2048, 512, 1024, 64
K, NH, DH, WIN, L = 4, 8, 64, 256, 2
T = M + S                      # 2112
CHUNK = 512                    # owned query tokens per core
HALO = 384                     # left halo in X-token space (keys start at local 128)
XLEN = HALO + CHUNK            # 896
NKEY = 768                     # key positions per core (local tokens 128..896)
NQ = 512
P = 128
NCORES = 8

LAST_RESULTS = None


def _build_nc(repeat=1):
    nc = bacc.Bacc("TRN2", target_bir_lowering=False, debug=False,
                   enable_asserts=False)

    def din(name, shape, dt):
        return nc.dram_tensor(name, shape, dt, kind="ExternalInput").ap()

    xt = din("xt", (4, P, XLEN), DT.bfloat16)
    msk = din("msk", (P, 4, 3, P), DT.bfloat16)
    qp_w = din("qp_w", (4, P, D), DT.bfloat16)
    kdiag = din("kdiag", (4, K, P, P), DT.bfloat16)
    qp_b = din("qp_b", (P, 4), DT.float32)
    w_in = din("w_in", (4, P, HID), DT.bfloat16)
    b_in = din("b_in", (P, 8), DT.float32)
    w_hid = din("w_hid", (L, 8, P, HID), DT.bfloat16)
    b_hid = din("b_hid", (P, L, 8), DT.float32)
    w_out = din("w_out", (8, P, D), DT.bfloat16)
    b_out = din("b_out", (P, 4), DT.float32)
    wq = din("wq", (4, P, D), DT.bfloat16)
    bq = din("bq", (P, 4), DT.float32)
    wk = din("wk", (4, P, D), DT.bfloat16)
    bk = din("bk", (P, 4), DT.float32)
    wv = din("wv", (4, P, D), DT.bfloat16)
    bvb = din("bvb", (1, D), DT.bfloat16)
    wo = din("wo", (4, P, D), DT.bfloat16)
    bo = din("bo", (P, 4), DT.float32)
    out = nc.dram_tensor("out", (4, P, NQ), DT.float32, kind="ExternalOutput").ap()

    aps = {k: v for k, v in locals().items() if k not in ('nc', 'din')}
    with tile.TileContext(nc) as tc:
        for _ in range(repeat):
            with ExitStack() as ctx:
                _body(ctx, tc, aps)
    nc.finalize()
    return nc


def _body(ctx, tc, aps):
    nc = tc.nc
    f32 = DT.float32
    b16 = DT.bfloat16

    consts = ctx.enter_context(tc.tile_pool(name="consts", bufs=1))
    acts = ctx.enter_context(tc.tile_pool(name="acts", bufs=1))
    work = ctx.enter_context(tc.tile_pool(name="work", bufs=4))

    dma = nc.sync.dma_start
    mm = nc.tensor.matmul

    # ---- stream inputs into SBUF ----
    xt_sb = acts.tile([P, 4, XLEN], b16, name="xt_sb")
    qp_w_sb = consts.tile([P, 4, D], b16, name="qp_w_sb")
    for cc in range(4):
        dma(out=xt_sb[:, cc], in_=aps['xt'][cc])
        dma(out=qp_w_sb[:, cc], in_=aps['qp_w'][cc])
    kdiag_sb = consts.tile([P, 4, K, P], b16, name="kdiag_sb")
    for cc in range(4):
        dma(out=kdiag_sb[:, cc], in_=aps['kdiag'][cc].rearrange("k p n -> p k n"))
    qp_b_sb = consts.tile([P, 4], f32, name="qp_b_sb")
    dma(out=qp_b_sb, in_=aps['qp_b'])
    w_in_sb = consts.tile([P, 4, HID], b16, name="w_in_sb")
    for cc in range(4):
        dma(out=w_in_sb[:, cc], in_=aps['w_in'][cc])
    b_in_sb = consts.tile([P, 8], f32, name="b_in_sb")
    dma(out=b_in_sb, in_=aps['b_in'])
    w_hid_sb = consts.tile([P, L, 8, HID], b16, name="w_hid_sb")
    for l in range(L):
        for cc in range(8):
            dma(out=w_hid_sb[:, l, cc], in_=aps['w_hid'][l, cc])
    b_hid_sb = consts.tile([P, L, 8], f32, name="b_hid_sb")
    dma(out=b_hid_sb, in_=aps['b_hid'])
    w_out_sb = consts.tile([P, 8, D], b16, name="w_out_sb")
    for cc in range(8):
        dma(out=w_out_sb[:, cc], in_=aps['w_out'][cc])
    b_out_sb = consts.tile([P, 4], f32, name="b_out_sb")
    dma(out=b_out_sb, in_=aps['b_out'])
    wq_sb = consts.tile([P, 4, D], b16, name="wq_sb")
    wk_sb = consts.tile([P, 4, D], b16, name="wk_sb")
    wv_sb = consts.tile([P, 4, D], b16, name="wv_sb")
    wo_sb = consts.tile([P, 4, D], b16, name="wo_sb")
    for cc in range(4):
        dma(out=wq_sb[:, cc], in_=aps['wq'][cc])
        dma(out=wk_sb[:, cc], in_=aps['wk'][cc])
        dma(out=wv_sb[:, cc], in_=aps['wv'][cc])
        dma(out=wo_sb[:, cc], in_=aps['wo'][cc])
    bq_sb = consts.tile([P, 4], f32, name="bq_sb")
    dma(out=bq_sb, in_=aps['bq'])
    bk_sb = consts.tile([P, 4], f32, name="bk_sb")
    dma(out=bk_sb, in_=aps['bk'])
    bvb_sb = consts.tile([1, D], b16, name="bvb_sb")
    dma(out=bvb_sb, in_=aps['bvb'])
    bo_sb = consts.tile([P, 4], f32, name="bo_sb")
    dma(out=bo_sb, in_=aps['bo'])
    mskt_sb = consts.tile([P, 4, 3, P], b16, name="mskt_sb")
    dma(out=mskt_sb, in_=aps['msk'])

    ident_sb = consts.tile([P, P], b16, name="ident_sb")
    make_identity(nc, ident_sb)
    ones_col = consts.tile([P, 1], b16, name="ones_col")
    nc.vector.memset(ones_col, 1.0)
    ones_row = consts.tile([1, P], b16, name="ones_row")
    nc.vector.memset(ones_row, 1.0)
    ones_row_f = consts.tile([1, P], f32, name="ones_row_f")
    nc.vector.memset(ones_row_f, 1.0)
    eps_sb = consts.tile([1, 1], f32, name="eps_sb")
    nc.vector.memset(eps_sb, 1e-24)

    # persistent activations
    qlin_sb = acts.tile([P, 4, XLEN], b16, name="qlin_sb")
    rq_sb = acts.tile([P, 4, NKEY], b16, name="rq_sb")
    sq_sb = acts.tile([P, 4, NKEY], b16, name="sq_sb")
    rqn_sb = acts.tile([P, 4, NKEY], b16, name="rqn_sb")
    n_sb = acts.tile([1, NKEY], f32, name="n_sb")
    nbc_sb = acts.tile([P, NKEY], b16, name="nbc_sb")
    h_a = acts.tile([P, 8, NKEY], b16, name="h_a")
    h_b = acts.tile([P, 8, NKEY], b16, name="h_b")
    r_sb = acts.tile([P, 4, NKEY], b16, name="r_sb")
    qt_sb = acts.tile([P, 4, NQ], b16, name="qt_sb")
    kt_sb = acts.tile([P, 4, NKEY], b16, name="kt_sb")
    v2_sb = acts.tile([P, 6, NH, DH + 1], b16, name="v2_sb")
    av_tp_sb = acts.tile([P, 4, D], b16, name="av_tp_sb")
    ao_sb = acts.tile([P, 4, NQ], b16, name="ao_sb")
    nc.vector.memset(v2_sb[:, :, :, DH:DH + 1], 1.0)

    with tc.tile_pool(name="ps2", bufs=3, space="PSUM") as ps2, \
         tc.tile_pool(name="ps1", bufs=2, space="PSUM") as ps1:

        def p2(name):
            # two PSUM banks; each [:, tt, :384] matmul slice is bank-aligned
            return ps2.tile([P, 2, 512], f32, tag="ps2", name=name)

        def p1(name, shape=(P, NQ)):
            return ps1.tile(list(shape), f32, tag="ps1", name=name)

        # ---- Phase A+B interleaved per channel-chunk: q_lin then conv ----
        # q_lin only needed for tokens [125:896) (conv taps); tile split at 512
        # so conv tt=0 depends only on the first tile.
        for dt_ in range(4):
            for tt, (t0, t1) in enumerate(((125, 512), (512, 896))):
                qlin_ps = p1("qlin_ps")
                for cc in range(4):
                    mm(qlin_ps[:, :t1 - t0],
                       qp_w_sb[:, cc, 128 * dt_:128 * (dt_ + 1)],
                       xt_sb[:, cc, t0:t1], start=(cc == 0), stop=(cc == 3))
                nc.vector.tensor_scalar_add(qlin_sb[:, dt_, t0:t1],
                                            qlin_ps[:, :t1 - t0],
                                            qp_b_sb[:, dt_:dt_ + 1])
            # conv for this channel chunk (key j = local token 128+j); copy and
            # square per token-tile so the norm chain starts during tt=1 conv
            cc = dt_
            rq_ps = p2("rq_ps")
            for tt in range(2):
                for k in range(K):
                    base = 125 + k + 384 * tt
                    mm(rq_ps[:, tt, :384], kdiag_sb[:, cc, k],
                       qlin_sb[:, cc, base:base + 384],
                       start=(k == 0), stop=(k == 3))
            nc.scalar.activation(rq_sb[:, cc], rq_ps[:, :, :384], AF.Copy)
            for tt in range(2):
                tsl = slice(384 * tt, 384 * (tt + 1))
                nc.vector.tensor_mul(sq_sb[:, cc, tsl], rq_sb[:, cc, tsl],
                                     rq_sb[:, cc, tsl])
        for tt in range(2):
            tsl = slice(384 * tt, 384 * (tt + 1))
            ss_ps = p1("ss_ps", (1, 384))
            for cc in range(4):
                mm(ss_ps, ones_col, sq_sb[:, cc, tsl],
                   start=(cc == 0), stop=(cc == 3))
            # sqrt(ss + 1e-24) == max(||q||, 1e-12) up to fp32 rounding; keeps
            # zero-padded halo tokens finite without a separate clamp op
            nc.scalar.activation(n_sb[:, tsl], ss_ps, AF.Sqrt, bias=eps_sb)
            nc.vector.reciprocal(n_sb[:, tsl], n_sb[:, tsl])
            nbc_ps = p1("nbc_ps", (P, 384))
            mm(nbc_ps, ones_row_f, n_sb[:, tsl], start=True, stop=True)
            nc.vector.tensor_copy(nbc_sb[:, tsl], nbc_ps)
            for cc in range(4):
                nc.vector.tensor_mul(rqn_sb[:, cc, tsl], rq_sb[:, cc, tsl],
                                     nbc_sb[:, tsl])

        # ---- Phase D: ResLinear MLP (768-wide point ops) ----
        for j in range(8):
            mi_ps = p2("mi_ps")
            for tt in range(2):
                tsl = slice(384 * tt, 384 * (tt + 1))
                for cc in range(4):
                    mm(mi_ps[:, tt, :384], w_in_sb[:, cc, 128 * j:128 * (j + 1)],
                       rqn_sb[:, cc, tsl], start=(cc == 0), stop=(cc == 3))
            nc.scalar.activation(h_a[:, j], mi_ps[:, :, :384],
                                 AF.Silu, bias=b_in_sb[:, j:j + 1])
        for l in range(L):
            src, dst = (h_a, h_b) if l == 0 else (h_b, h_a)
            for j in range(8):
                mh_ps = p2("mh_ps")
                for tt in range(2):
                    tsl = slice(384 * tt, 384 * (tt + 1))
                    for cc in range(8):
                        mm(mh_ps[:, tt, :384],
                           w_hid_sb[:, l, cc, 128 * j:128 * (j + 1)],
                           src[:, cc, tsl], start=(cc == 0), stop=(cc == 7))
                slu = work.tile([P, NKEY], b16, name="slu")
                nc.scalar.activation(slu, mh_ps[:, :, :384],
                                     AF.Silu, bias=b_hid_sb[:, l, j:j + 1])
                nc.vector.tensor_add(dst[:, j], src[:, j], slu)
        h_fin = h_a
        for dt_ in range(4):
            mo_ps = p2("mo_ps")
            for tt in range(2):
                tsl = slice(384 * tt, 384 * (tt + 1))
                for cc in range(8):
                    mm(mo_ps[:, tt, :384], w_out_sb[:, cc, 128 * dt_:128 * (dt_ + 1)],
                       h_fin[:, cc, tsl], start=(cc == 0), stop=(cc == 7))
            nc.scalar.activation(r_sb[:, dt_], mo_ps[:, :, :384],
                                 AF.Silu, bias=b_out_sb[:, dt_:dt_ + 1])

        # ---- Phase E: QKV projections ----
        for dt_ in range(4):
            q_ps = p1("q_ps")
            for cc in range(4):
                mm(q_ps, wq_sb[:, cc, 128 * dt_:128 * (dt_ + 1)],
                   r_sb[:, cc, 256:768], start=(cc == 0), stop=(cc == 3))
            nc.vector.tensor_scalar_add(qt_sb[:, dt_], q_ps, bq_sb[:, dt_:dt_ + 1])
            k_ps = p2("k_ps")
            for tt in range(2):
                tsl = slice(384 * tt, 384 * (tt + 1))
                for cc in range(4):
                    mm(k_ps[:, tt, :384], wk_sb[:, cc, 128 * dt_:128 * (dt_ + 1)],
                       r_sb[:, cc, tsl], start=(cc == 0), stop=(cc == 3))
            nc.vector.tensor_scalar_add(kt_sb[:, dt_], k_ps[:, :, :384],
                                        bk_sb[:, dt_:dt_ + 1])
        for tb in range(6):
            v_ps = p1("v_ps")
            mm(v_ps, ones_row, bvb_sb, start=True, stop=False)  # rank-1 bias
            for cc in range(4):
                mm(v_ps, r_sb[:, cc, 128 * tb:128 * (tb + 1)], wv_sb[:, cc],
                   start=False, stop=(cc == 3))
            nc.vector.tensor_copy(v2_sb[:, tb, :, :DH],
                                  v_ps.rearrange("p (h d) -> p h d", h=NH))

    # ---- Phase F: attention with transposed scores (k on partitions) ----
    # sT[k, q] = K_h Q_h^T + mask^T; p = exp(sT/8); AV uses p-chunks as lhsT so
    # the output lands token-partitioned, with a 65th ones-column of V giving
    # the softmax denominator for free. Normalization happens on the AV output
    # (per-partition scalars), then a final PE transpose back to CP layout.
    with tc.tile_pool(name="psc", bufs=4, space="PSUM") as psc, \
         tc.tile_pool(name="psav", bufs=2, space="PSUM") as psav, \
         tc.tile_pool(name="pstr", bufs=2, space="PSUM") as pstr:
        for qb in range(4):

            def emit_scores(hp, qb=qb):
                tiles = []
                for i in (0, 1):
                    pbase = 64 * i
                    st_ps = psc.tile([P, 3, P], DT.float32, tag="sc", name="st_ps")
                    for kc in range(3):
                        # kc==1 covers keys [q-127, q+128) minus edges: fully
                        # inside the causal window for every q; only qb==0 can
                        # clip it at the sequence start (kmin).
                        need_mask = (kc != 1) or (qb == 0)
                        if need_mask:
                            mm(st_ps[:, kc, :], ident_sb, mskt_sb[:, qb, kc],
                               start=True, stop=False)
                        mm(st_ps[:, kc, :],
                           kt_sb[pbase:pbase + 64, hp, 128 * (qb + kc):128 * (qb + kc + 1)],
                           qt_sb[pbase:pbase + 64, hp, 128 * qb:128 * (qb + 1)],
                           start=(not need_mask), stop=True)
                    tiles.append(st_ps)
                return tiles

            def emit_softmax_av(hp, tiles, qb=qb):
                for i in (0, 1):
                    h = 2 * hp + i
                    pts_sb = work.tile([P, 384], DT.bfloat16, name="pts_sb")
                    nc.scalar.activation(pts_sb, tiles[i], AF.Exp, scale=0.125)
                    av_ps = psav.tile([P, DH + 1], DT.float32, tag="av", name="av_ps")
                    for kc in range(3):
                        mm(av_ps, pts_sb[:, 128 * kc:128 * (kc + 1)],
                           v2_sb[:, qb + kc, h, :],
                           start=(kc == 0), stop=(kc == 2))
                    rs = work.tile([P, 1], DT.float32, name="rs")
                    nc.vector.reciprocal(rs, av_ps[:, DH:DH + 1])
                    nc.vector.tensor_scalar_mul(av_tp_sb[:, qb, DH * h:DH * (h + 1)],
                                                av_ps[:, :DH], rs)

            pend = None
            for hp in range(4):
                tiles = emit_scores(hp)
                if pend is not None:
                    emit_softmax_av(*pend)
                pend = (hp, tiles)
            emit_softmax_av(*pend)
            # transpose attention output back to CP layout, then do this
            # query-block's slice of the output projection immediately so the
            # o-proj and output DMA overlap the remaining attention blocks
            tr_ps = pstr.tile([P, 4, P], DT.float32, tag="tr", name="tr_ps")
            for cc in range(4):
                mm(tr_ps[:, cc, :], av_tp_sb[:, qb, 128 * cc:128 * (cc + 1)],
                   ident_sb, start=True, stop=True)
            nc.vector.tensor_copy(ao_sb[:, :, 128 * qb:128 * (qb + 1)], tr_ps)
            for dt_ in range(4):
                o_ps = pstr.tile([P, P], DT.float32, tag="tr", name="o_ps")
                for cc in range(4):
                    mm(o_ps, wo_sb[:, cc, 128 * dt_:128 * (dt_ + 1)],
                       ao_sb[:, cc, 128 * qb:128 * (qb + 1)],
                       start=(cc == 0), stop=(cc == 3))
                o_sb = work.tile([P, P], DT.float32, name="o_sb")
                nc.vector.tensor_scalar_add(o_sb, o_ps, bo_sb[:, dt_:dt_ + 1])
                dma(out=aps['out'][dt_][:, 128 * qb:128 * (qb + 1)], in_=o_sb)


def _host_prep(inputs):
    f32 = np.float32
    x = inputs['x'].astype(f32)
    mmem = np.broadcast_to(inputs['meta_memory'].astype(f32)[None], (B, M, D))
    xm = np.concatenate([mmem, x], axis=1)  # [B, T, D]

    def chunkw(w, p=P):  # [n, m] -> [n//p, p, m]
        return np.ascontiguousarray(w.reshape(-1, p, w.shape[-1]))

    kern = inputs['qp_conv'].astype(f32)  # [K, D]
    kd = np.zeros((4, K, P, P), f32)
    for cc in range(4):
        for k in range(K):
            np.fill_diagonal(kd[cc, k], kern[k, 128 * cc:128 * (cc + 1)])

    shared = {
        'qp_w': chunkw(inputs['qp_w'].astype(bf16)),
        'kdiag': kd.astype(bf16),
        'qp_b': np.ascontiguousarray(inputs['qp_b'].astype(f32).reshape(4, P).T),
        'w_in': chunkw(inputs['mlp_w_in'].astype(bf16)),
        'b_in': np.ascontiguousarray(inputs['mlp_b_in'].astype(f32).reshape(8, P).T),
        'w_hid': np.ascontiguousarray(
            inputs['mlp_w_hid'].astype(bf16).reshape(L, 8, P, HID)),
        'b_hid': np.ascontiguousarray(
            inputs['mlp_b_hid'].astype(f32).reshape(L, 8, P).transpose(2, 0, 1)),
        'w_out': chunkw(inputs['mlp_w_out'].astype(bf16)),
        'b_out': np.ascontiguousarray(inputs['mlp_b_out'].astype(f32).reshape(4, P).T),
        'wq': chunkw(inputs['swa_wq'].astype(bf16)),
        'bq': np.ascontiguousarray(inputs['swa_bq'].astype(f32).reshape(4, P).T),
        'wk': chunkw(inputs['swa_wk'].astype(bf16)),
        'bk': np.ascontiguousarray(inputs['swa_bk'].astype(f32).reshape(4, P).T),
        'wv': chunkw(inputs['swa_wv'].astype(bf16)),
        'bvb': np.ascontiguousarray(inputs['swa_bv'].astype(bf16)[None, :]),
        'wo': chunkw(inputs['swa_wo'].astype(bf16)),
        'bo': np.ascontiguousarray(inputs['swa_bo'].astype(f32).reshape(4, P).T),
    }

    pp = np.arange(P)[:, None]
    cc = np.arange(384)[None, :]
    band = (cc - pp >= 1) & (cc - pp <= 256)
    in_maps = []
    for core in range(NCORES):
        b, c = divmod(core, 4)
        qg0 = M + c * CHUNK
        Xl = np.zeros((XLEN, D), f32)
        g0 = qg0 - HALO
        s0 = max(0, g0)
        Xl[s0 - g0:] = xm[b, s0:qg0 + CHUNK]
        xt = np.ascontiguousarray(Xl.T.reshape(4, P, XLEN)).astype(bf16)
        kmin = 192 if c == 0 else 0
        m = np.empty((4, P, 384), f32)
        for qb in range(4):
            allowed = band & (qb * P + cc >= kmin)
            m[qb] = np.where(allowed, 0.0, -8e9)
        # transpose to [k-within-chunk, qb, kc, q]
        mt = m.reshape(4, P, 3, P).transpose(3, 0, 2, 1)
        msk = np.ascontiguousarray(mt).astype(bf16)
        im = dict(shared)
        im['xt'] = xt
        im['msk'] = msk
        in_maps.append(im)
    return in_maps


_NC_CACHE = None
_HOOK_WRAPPED = False


def _wrap_cc_hook():
    """Surface the real exception when the PJRT python compile hook fails
    (otherwise it shows as an opaque CallFunctionObjArgs error)."""
    global _HOOK_WRAPPED
    if _HOOK_WRAPPED:
        return
    try:
        from concourse import bass2jax
        import libneuronxla
        import traceback
        import sys
        bass2jax.install_neuronx_cc_hook()
        orig = libneuronxla.neuronx_cc

        def wrapped(*a, **k):
            try:
                return orig(*a, **k)
            except Exception:
                traceback.print_exc(file=sys.stderr)
                sys.stderr.flush()
                raise

        libneuronxla.neuronx_cc = wrapped
        bass2jax.install_neuronx_cc_hook = lambda: None
        _HOOK_WRAPPED = True
    except ImportError:
        pass


def kernel(**inputs):
    _wrap_cc_hook()
    global _NC_CACHE, LAST_RESULTS
    inputs = {k: np.asarray(v) for k, v in inputs.items()}
    in_maps = _host_prep(inputs)
    if _NC_CACHE is None:
        _NC_CACHE = _build_nc()
    nc = _NC_CACHE
    res = run_bass_kernel_spmd(nc, in_maps, core_ids=list(range(NCORES)))
    LAST_RESULTS = res
    outf = np.zeros((B, S, D), np.float32)
    for core in range(NCORES):
        b, c = divmod(core, 4)
        o_cp = res.results[core]['out'].reshape(D, NQ)
        outf[b, c * CHUNK:(c + 1) * CHUNK, :] = o_cp.T
    return outf
